# revision 10
# baseline (speedup 1.0000x reference)
"""Channel-attention (XCA-style) Trainium2 kernel, 8-core SPMD.

Sharding: spatial row-bands (32 rows/core + 1-row halo), both batches on
every core. Cross-core coupling is only the per-(batch,head) Gram matrix
[q;k][q;k]^T and the q/k squared norms, all-reduced as one tiny [96,384]
fp32 buffer per batch. Softmax + proj-folding is recomputed redundantly
per core; the attention@v + proj matmul is fused into a single
PeffT.T @ v matmul per batch.

Pipeline per core, per batch:
  x band --1x1(PE)--> mid(+guard cols) --depthwise 3x3(DVE FMA)--> q,k
  q,k --DMA-xbar-transpose--> uT chunks --PE Gram--> Gu psum --AllReduce
  y band --1x1(PE)--> v1(+guards) --9-tap full 3x3 (PE)--> v
  post-AR: norms/softmax/Peff (tiny, redundant) -> PeffT
  final: PeffT.T @ v -> out band (fp32)
"""
import os
import sys

sys.path.insert(0, '/opt/trn_rl_repo')

import numpy as np
import ml_dtypes

import concourse.bass as bass
import concourse.bacc as bacc
import concourse.tile as tile
import concourse.mybir as mybir
from concourse.bass_utils import run_bass_kernel_spmd

BF16 = mybir.dt.bfloat16
F32 = mybir.dt.float32
ADD = mybir.AluOpType.add
MULT = mybir.AluOpType.mult

N_CORES = 8
DIM = 192
HEADS = 4
HD = DIM // HEADS            # 48
UD = 2 * HD                  # 96 channels per head in u=[q_h;k_h] order
B = 2
H = 256
W = 256
ROWS = H // N_CORES          # 32 output rows per core
RIN = ROWS + 2               # input band rows (halo)
WG = W + 2                   # guarded width
NI = ROWS * W                # 8192 interior positions per batch
NF = RIN * W                 # 8704 band positions per batch
NB = 512                     # matmul N-chunk
TAPS = [(dy, dx) for dy in (-1, 0, 1) for dx in (-1, 0, 1)]

# segments of qk tiles covering u-channels [96h, 96h+96) : (tile, lo, hi)
# XBAR transpose DMA sources starting at partition > 0 are limited to 32
# partitions, so non-zero-base segments are split into 32-row chunks.
def _segs(raw):
    out = []
    for (ct, lo, hi) in raw:
        if lo == 0:
            out.append((ct, lo, hi))
        else:
            out += [(ct, s, min(s + 32, hi)) for s in range(lo, hi, 32)]
    return out


HEAD_SEGS = {
    0: _segs([(0, 0, 96)]),
    1: _segs([(0, 96, 128), (1, 0, 64)]),
    2: _segs([(1, 64, 128), (2, 0, 32)]),
    3: _segs([(2, 32, 128)]),
}

LAST_RESULTS = None
_CACHED_NC = None


def _u_perm():
    p = []
    for h in range(HEADS):
        p += list(range(h * HD, (h + 1) * HD))
        p += list(range(DIM + h * HD, DIM + (h + 1) * HD))
    return np.array(p)


def _bf16(a):
    return np.ascontiguousarray(a.astype(ml_dtypes.bfloat16))


def build_nc():
    nc = bacc.Bacc("TRN2", target_bir_lowering=False, debug=False,
                   enable_asserts=False, num_devices=N_CORES)
    xb = nc.dram_tensor("xb", [B, DIM, NF], BF16, kind="ExternalInput").ap()
    yb = nc.dram_tensor("yb", [B, DIM, NF], BF16, kind="ExternalInput").ap()
    wqk = nc.dram_tensor("wqk", [DIM, 2 * DIM], BF16, kind="ExternalInput").ap()
    aqk = nc.dram_tensor("aqk", [3, 128, 9], F32, kind="ExternalInput").ap()
    wv = nc.dram_tensor("wv", [DIM, DIM], BF16, kind="ExternalInput").ap()
    wvdw = nc.dram_tensor("wvdw", [9, DIM, DIM], BF16, kind="ExternalInput").ap()
    projth = nc.dram_tensor("projth", [HEADS, HD, DIM], BF16, kind="ExternalInput").ap()
    tempt = nc.dram_tensor("tempt", [HD, HEADS], F32, kind="ExternalInput").ap()
    eye = nc.dram_tensor("eye", [UD, UD], F32, kind="ExternalInput").ap()
    out = nc.dram_tensor("out", [B, DIM, NI], F32, kind="ExternalOutput").ap()

    with tile.TileContext(nc) as tc:
        with tc.tile_pool(name="wpool", bufs=1) as wp, \
             tc.tile_pool(name="xy", bufs=1) as xyp, \
             tc.tile_pool(name="mid", bufs=1) as midp, \
             tc.tile_pool(name="qk", bufs=1) as qkp, \
             tc.tile_pool(name="ut", bufs=3) as utp, \
             tc.tile_pool(name="vv", bufs=1) as vvp, \
             tc.tile_pool(name="small", bufs=1) as smp, \
             tc.tile_pool(name="ost", bufs=3) as ostp, \
             tc.tile_pool(name="psA", bufs=3, space="PSUM") as psA, \
             tc.tile_pool(name="psB", bufs=2, space="PSUM") as psB, \
             tc.tile_pool(name="psG", bufs=2, space="PSUM") as psG, \
             tc.tile_pool(name="psS", bufs=1, space="PSUM") as psS, \
             tc.tile_pool(name="dram", bufs=2, space="DRAM") as drp:

            # ---- persistent weights ----
            wqk_a = wp.tile([128, 2 * DIM], BF16, tag="wqk_a")
            wqk_b = wp.tile([64, 2 * DIM], BF16, tag="wqk_b")
            nc.sync.dma_start(wqk_a[:], wqk[0:128, :])
            nc.sync.dma_start(wqk_b[:], wqk[128:192, :])
            wv_a = wp.tile([128, DIM], BF16, tag="wv_a")
            wv_b = wp.tile([64, DIM], BF16, tag="wv_b")
            nc.sync.dma_start(wv_a[:], wv[0:128, :])
            nc.sync.dma_start(wv_b[:], wv[128:192, :])
            wvdw_a = wp.tile([128, 9, DIM], BF16, tag="wvdw_a")
            wvdw_b = wp.tile([64, 9, DIM], BF16, tag="wvdw_b")
            nc.sync.dma_start(wvdw_a[:], wvdw[:, 0:128, :].rearrange("t k m -> k t m"))
            nc.sync.dma_start(wvdw_b[:], wvdw[:, 128:192, :].rearrange("t k m -> k t m"))
            aqk_sb = wp.tile([128, 3, 9], F32, tag="aqk")
            nc.sync.dma_start(aqk_sb[:], aqk.rearrange("c k t -> k c t"))
            projth_sb = wp.tile([HD, HEADS, DIM], BF16, tag="projth")
            nc.sync.dma_start(projth_sb[:], projth.rearrange("h d f -> d h f"))
            tempt_sb = wp.tile([HD, HEADS], F32, tag="tempt")
            nc.sync.dma_start(tempt_sb[:], tempt[:])
            eye_sb = wp.tile([UD, UD], F32, tag="eye")
            nc.sync.dma_start(eye_sb[:], eye[:])

            for b in range(B):
                # ================= QK path =================
                x_a = xyp.tile([128, NF], BF16, tag="band_a")
                x_b = xyp.tile([64, NF], BF16, tag="band_b")
                nc.sync.dma_start(x_a[:], xb[b, 0:128, :])
                nc.sync.dma_start(x_b[:], xb[b, 128:192, :])

                qkt = []
                for ct in range(3):
                    mid = midp.tile([128, RIN, WG], BF16, tag="mid")
                    mid2 = midp.tile([128, RIN, W], BF16, tag="mid2")
                    nc.vector.memset(mid[:, :, 0:1], 0.0)
                    nc.vector.memset(mid[:, :, WG - 1:WG], 0.0)
                    mcol = slice(ct * 128, (ct + 1) * 128)
                    for nb in range(NF // NB):
                        ns = slice(nb * NB, (nb + 1) * NB)
                        ps = psA.tile([128, NB], F32, tag="psA")
                        nc.tensor.matmul(ps[:], wqk_a[:, mcol], x_a[:, ns],
                                         start=True, stop=False)
                        nc.tensor.matmul(ps[:], wqk_b[:, mcol], x_b[:, ns],
                                         start=False, stop=True)
                        r = nb * 2
                        psv = ps[:].rearrange("p (r w) -> p r w", r=2)
                        nc.scalar.copy(mid[:, r:r + 2, 1:W + 1], psv)
                        nc.vector.tensor_copy(mid2[:, r:r + 2, :], psv)
                    # depthwise 3x3 -> q,k interior.
                    # tensor_scalar runs at 4x and tensor_tensor at 2x on
                    # DVE, while the fused scalar_tensor_tensor is stuck at
                    # 1x — so emit products + adds separately.
                    qt = qkp.tile([128, ROWS, W], BF16, tag=f"qk{ct}")
                    dwt = qkp.tile([128, ROWS, W], BF16, tag="dwtmp")
                    nc.vector.tensor_scalar_mul(
                        qt[:], mid2[:, 1:1 + ROWS, :], aqk_sb[:, ct, 4:5])
                    for t, (dy, dx) in enumerate(TAPS):
                        if (dy, dx) == (0, 0):
                            continue
                        rs = slice(1 + dy, 1 + ROWS + dy)
                        if dx == 0:
                            src = mid2[:, rs, :]
                        else:
                            src = mid[:, rs, 1 + dx:1 + W + dx]
                        nc.vector.tensor_scalar_mul(
                            dwt[:], src, aqk_sb[:, ct, t:t + 1])
                        nc.vector.tensor_tensor(qt[:], dwt[:], qt[:], op=ADD)
                    qkt.append(qt)

                # ================= V path =================
                y_a = xyp.tile([128, NF], BF16, tag="band_a")
                y_b = xyp.tile([64, NF], BF16, tag="band_b")
                nc.sync.dma_start(y_a[:], yb[b, 0:128, :])
                nc.sync.dma_start(y_b[:], yb[b, 128:192, :])
                v1a = midp.tile([128, RIN, WG], BF16, tag="mid")
                v1b = midp.tile([64, RIN, WG], BF16, tag="mid2")
                for t_ in (v1a, v1b):
                    nc.vector.memset(t_[:, :, 0:1], 0.0)
                    nc.vector.memset(t_[:, :, WG - 1:WG], 0.0)
                for nb in range(NF // NB):
                    ns = slice(nb * NB, (nb + 1) * NB)
                    ps = psA.tile([128, NB], F32, tag="psA")
                    ps2 = psB.tile([64, NB], F32, tag="psB")
                    nc.tensor.matmul(ps[:], wv_a[:, 0:128], y_a[:, ns],
                                     start=True, stop=False)
                    nc.tensor.matmul(ps[:], wv_b[:, 0:128], y_b[:, ns],
                                     start=False, stop=True)
                    nc.tensor.matmul(ps2[:], wv_a[:, 128:192], y_a[:, ns],
                                     start=True, stop=False)
                    nc.tensor.matmul(ps2[:], wv_b[:, 128:192], y_b[:, ns],
                                     start=False, stop=True)
                    r = nb * 2
                    nc.scalar.copy(v1a[:, r:r + 2, 1:W + 1],
                                   ps[:].rearrange("p (r w) -> p r w", r=2))
                    nc.scalar.copy(v1b[:, r:r + 2, 1:W + 1],
                                   ps2[:].rearrange("p (r w) -> p r w", r=2))
                # full 3x3 conv, 9 shifted matmuls
                v0 = vvp.tile([128, NI], BF16, tag="v0")
                vB = vvp.tile([64, NI], BF16, tag="vB")
                for nb in range(NI // NB):
                    ns = slice(nb * NB, (nb + 1) * NB)
                    r = nb * 2
                    psv = psA.tile([128, NB], F32, tag="psA")
                    psv2 = psB.tile([64, NB], F32, tag="psB")
                    for t, (dy, dx) in enumerate(TAPS):
                        rs = slice(1 + r + dy, 3 + r + dy)
                        cs = slice(1 + dx, 1 + W + dx)
                        rhsA = v1a[:, rs, cs]
                        rhsB = v1b[:, rs, cs]
                        nc.tensor.matmul(psv[:], wvdw_a[:, t, 0:128], rhsA,
                                         start=(t == 0), stop=False)
                        nc.tensor.matmul(psv[:], wvdw_b[:, t, 0:128], rhsB,
                                         start=False, stop=(t == 8))
                        nc.tensor.matmul(psv2[:], wvdw_a[:, t, 128:192], rhsA,
                                         start=(t == 0), stop=False)
                        nc.tensor.matmul(psv2[:], wvdw_b[:, t, 128:192], rhsB,
                                         start=False, stop=(t == 8))
                    nc.vector.tensor_copy(v0[:, ns], psv[:])
                    nc.vector.tensor_copy(vB[:, ns], psv2[:])

                # ================= Gram =================
                gu_all = smp.tile([UD, HEADS, UD], F32, tag="gu_all")
                n_bands = NI // 2048
                for h in range(HEADS):
                    gps = psG.tile([UD, UD], F32, tag="psG")
                    for band in range(n_bands):
                        rsl = slice(band * 8, (band + 1) * 8)
                        ut = utp.tile([128, 16, UD], BF16, tag="ut")
                        off = 0
                        for (ct, lo, hi) in HEAD_SEGS[h]:
                            nc.sync.dma_start_transpose(
                                ut[:, :, off:off + hi - lo],
                                qkt[ct][lo:hi, rsl, :])
                            off += hi - lo
                        for c in range(16):
                            nc.tensor.matmul(
                                gps[:], ut[:, c, :], ut[:, c, :],
                                start=(band == 0 and c == 0),
                                stop=(band == n_bands - 1 and c == 15))
                    nc.vector.tensor_copy(gu_all[:, h, :], gps[:])

                # ================= AllReduce =================
                gu_in = drp.tile([UD, HEADS * UD], F32, tag="gu_in")
                gu_out = drp.tile([UD, HEADS * UD], F32, tag="gu_out")
                nc.sync.dma_start(gu_in[:], gu_all[:].rearrange("p h d -> p (h d)"))
                nc.gpsimd.collective_compute(
                    "AllReduce", ADD,
                    replica_groups=[list(range(N_CORES))],
                    ins=[gu_in.opt()], outs=[gu_out.opt()])
                gur = smp.tile([UD, HEADS, UD], F32, tag="gur")
                nc.sync.dma_start(gur[:].rearrange("p h d -> p (h d)"), gu_out[:])

                # ================= post-AR smalls =================
                tmp = smp.tile([UD, UD], F32, tag="tmp")
                nsq = smp.tile([UD, HEADS], F32, tag="nsq")
                ksqr = smp.tile([1, HEADS, UD], F32, tag="ksqr")
                for h in range(HEADS):
                    # masked diag of the whole [96,96] head block (base 0:
                    # partition bases must be 32-aligned)
                    nc.vector.tensor_tensor(
                        tmp[:], gur[:, h, :], eye_sb[:], op=MULT)
                    # column form [96,1]: rows 0:48 hold q norms^2
                    nc.vector.tensor_reduce(
                        nsq[:, h:h + 1], tmp[:],
                        axis=mybir.AxisListType.X, op=ADD)
                    # row form [1,96]: cols 48:96 hold k norms^2
                    nc.gpsimd.tensor_reduce(
                        ksqr[:, h, :], tmp[:],
                        axis=mybir.AxisListType.C, op=ADD)
                rn = smp.tile([HD, HEADS], F32, tag="rn")
                inv = smp.tile([HD, HEADS], F32, tag="inv")
                nc.scalar.sqrt(rn[:], nsq[0:HD, :])
                nc.vector.reciprocal(inv[:], rn[:])
                invq = smp.tile([HD, HEADS], F32, tag="invq")
                nc.vector.tensor_tensor(invq[:], inv[:], tempt_sb[:], op=MULT)
                krn = smp.tile([1, HEADS, UD], F32, tag="krn")
                invkr = smp.tile([1, HEADS, UD], F32, tag="invkr")
                nc.scalar.sqrt(krn[:], ksqr[:])
                nc.vector.reciprocal(invkr[:], krn[:])
                mst = smp.tile([HD, HEADS, DIM], BF16, tag="mst")
                for h in range(HEADS):
                    bc = smp.tile([HD, HD], F32, tag="bc")
                    nc.gpsimd.partition_broadcast(bc[:], invkr[:, h, HD:UD])
                    lg = smp.tile([HD, HD], F32, tag="lg")
                    nc.vector.tensor_scalar_mul(
                        lg[:], gur[0:HD, h, HD:UD], invq[:, h:h + 1])
                    nc.vector.tensor_tensor(lg[:], lg[:], bc[:], op=MULT)
                    nm = smp.tile([HD, 1], F32, tag="nm")
                    nc.vector.tensor_reduce(
                        nm[:], lg[:], axis=mybir.AxisListType.X,
                        op=mybir.AluOpType.max, negate=True)
                    ex = smp.tile([HD, HD], F32, tag="ex")
                    nc.scalar.activation(ex[:], lg[:],
                                         mybir.ActivationFunctionType.Exp,
                                         bias=nm[:], scale=1.0)
                    sm = smp.tile([HD, 1], F32, tag="sm")
                    nc.vector.tensor_reduce(sm[:], ex[:],
                                            axis=mybir.AxisListType.X, op=ADD)
                    rs_ = smp.tile([HD, 1], F32, tag="rs_")
                    nc.vector.reciprocal(rs_[:], sm[:])
                    at = smp.tile([HD, HD], BF16, tag="at")
                    nc.vector.tensor_scalar_mul(at[:], ex[:], rs_[:])
                    mps = psS.tile([HD, DIM], F32, tag="psS")
                    nc.tensor.matmul(mps[:], at[:], projth_sb[:, h, :],
                                     start=True, stop=True)
                    nc.vector.tensor_copy(mst[:, h, :], mps[:])
                peff_a = smp.tile([128, DIM], BF16, tag="peff_a")
                peff_b = smp.tile([64, DIM], BF16, tag="peff_b")
                nc.sync.dma_start(peff_a[0:48, :], mst[:, 0, :])
                nc.sync.dma_start(peff_a[48:96, :], mst[:, 1, :])
                nc.sync.dma_start(peff_a[96:128, :], mst[0:32, 2, :])
                nc.sync.dma_start(peff_b[0:16, :], mst[32:48, 2, :])
                nc.sync.dma_start(peff_b[16:64, :], mst[:, 3, :])

                # ================= final =================
                for (f0, f1) in ((0, 128), (128, 192)):
                    for nb in range(NI // NB):
                        ns = slice(nb * NB, (nb + 1) * NB)
                        psf = (psA if f1 - f0 == 128 else psB).tile(
                            [f1 - f0, NB], F32, tag="psA" if f1 - f0 == 128 else "psB")
                        nc.tensor.matmul(psf[:], peff_a[:, f0:f1], v0[:, ns],
                                         start=True, stop=False)
                        nc.tensor.matmul(psf[:], peff_b[:, f0:f1], vB[:, ns],
                                         start=False, stop=True)
                        ost = ostp.tile([f1 - f0, NB], F32, tag="ost")
                        nc.scalar.copy(ost[:], psf[:])
                        nc.sync.dma_start(out[b, f0:f1, ns], ost[:])

    nc.finalize()
    return nc


def _host_prep(inputs):
    x = np.asarray(inputs["x"], dtype=np.float32)
    y = np.asarray(inputs["y"], dtype=np.float32)
    qk_w = np.asarray(inputs["qk_w"], dtype=np.float32)[:, :, 0, 0]
    qk_dw = np.asarray(inputs["qk_dw_w"], dtype=np.float32)[:, 0]
    v_w = np.asarray(inputs["v_w"], dtype=np.float32)[:, :, 0, 0]
    v_dw = np.asarray(inputs["v_dw_w"], dtype=np.float32)
    proj = np.asarray(inputs["proj_w"], dtype=np.float32)[:, :, 0, 0]
    temp = np.asarray(inputs["temperature"], dtype=np.float32).reshape(HEADS)

    perm = _u_perm()
    wqk_l = _bf16(qk_w[perm].T)                              # [192, 384]
    aqk_t = np.ascontiguousarray(qk_dw[perm].reshape(3, 128, 9).astype(np.float32))
    wv_l = _bf16(v_w.T)                                      # [192, 192]
    wvdw_l = _bf16(np.stack(
        [v_dw[:, :, dy + 1, dx + 1].T for dy, dx in TAPS]))  # [9,192,192]
    projth = _bf16(np.stack(
        [proj[:, h * HD:(h + 1) * HD].T for h in range(HEADS)]))  # [4,48,192]
    tempt = np.ascontiguousarray(
        np.broadcast_to(temp[None, :], (HD, HEADS)).astype(np.float32))
    eye = np.eye(UD, dtype=np.float32)

    # halo-padded row bands per core, bf16
    xp = np.pad(x, ((0, 0), (0, 0), (1, 1), (0, 0)))
    yp = np.pad(y, ((0, 0), (0, 0), (1, 1), (0, 0)))
    shared = dict(wqk=wqk_l, aqk=aqk_t, wv=wv_l, wvdw=wvdw_l,
                  projth=projth, tempt=tempt, eye=eye)
    in_maps = []
    for c in range(N_CORES):
        rs = slice(c * ROWS, c * ROWS + RIN)
        in_maps.append(dict(
            xb=_bf16(xp[:, :, rs]).reshape(B, DIM, NF),
            yb=_bf16(yp[:, :, rs]).reshape(B, DIM, NF),
            **shared))
    return in_maps


def kernel(**inputs):
    global LAST_RESULTS, _CACHED_NC
    in_maps = _host_prep(inputs)
    if _CACHED_NC is None:
        _CACHED_NC = build_nc()
    res = run_bass_kernel_spmd(
        _CACHED_NC, in_maps, core_ids=list(range(N_CORES)))
    LAST_RESULTS = res
    out = np.empty((B, DIM, H, W), np.float32)
    for c in range(N_CORES):
        band = res.results[c]["out"].reshape(B, DIM, ROWS, W)
        out[:, :, c * ROWS:(c + 1) * ROWS] = band
    return out


# revision 12
# speedup vs baseline: 1.0824x; 1.0824x over previous
"""Channel-attention (XCA-style) Trainium2 kernel, 8-core SPMD.

Sharding: spatial row-bands (32 rows/core + 1-row halo), both batches on
every core. Cross-core coupling is only the per-(batch,head) Gram matrix
[q;k][q;k]^T and the q/k squared norms, all-reduced as one tiny [96,384]
fp32 buffer per batch. Softmax + proj-folding is recomputed redundantly
per core; the attention@v + proj matmul is fused into a single
PeffT.T @ v matmul per batch.

Pipeline per core, per batch:
  x band --1x1(PE)--> mid(+guard cols) --depthwise 3x3(DVE FMA)--> q,k
  q,k --DMA-xbar-transpose--> uT chunks --PE Gram--> Gu psum --AllReduce
  y band --1x1(PE)--> v1(+guards) --9-tap full 3x3 (PE)--> v
  post-AR: norms/softmax/Peff (tiny, redundant) -> PeffT
  final: PeffT.T @ v -> out band (fp32)
"""
import os
import sys

sys.path.insert(0, '/opt/trn_rl_repo')

import numpy as np
import ml_dtypes

import concourse.bass as bass
import concourse.bacc as bacc
import concourse.tile as tile
import concourse.mybir as mybir
from concourse.bass_utils import run_bass_kernel_spmd

BF16 = mybir.dt.bfloat16
F32 = mybir.dt.float32
ADD = mybir.AluOpType.add
MULT = mybir.AluOpType.mult

N_CORES = 8
DIM = 192
HEADS = 4
HD = DIM // HEADS            # 48
UD = 2 * HD                  # 96 channels per head in u=[q_h;k_h] order
B = 2
H = 256
W = 256
ROWS = H // N_CORES          # 32 output rows per core
RIN = ROWS + 2               # input band rows (halo)
WG = W + 2                   # guarded width
NI = ROWS * W                # 8192 interior positions per batch
NF = RIN * W                 # 8704 band positions per batch
NB = 512                     # matmul N-chunk
TAPS = [(dy, dx) for dy in (-1, 0, 1) for dx in (-1, 0, 1)]

# segments of qk tiles covering u-channels [96h, 96h+96) : (tile, lo, hi)
# XBAR transpose DMA sources starting at partition > 0 are limited to 32
# partitions, so non-zero-base segments are split into 32-row chunks.
def _segs(raw):
    out = []
    for (ct, lo, hi) in raw:
        if lo == 0:
            out.append((ct, lo, hi))
        else:
            out += [(ct, s, min(s + 32, hi)) for s in range(lo, hi, 32)]
    return out


HEAD_SEGS = {
    0: _segs([(0, 0, 96)]),
    1: _segs([(0, 96, 128), (1, 0, 64)]),
    2: _segs([(1, 64, 128), (2, 0, 32)]),
    3: _segs([(2, 32, 128)]),
}

LAST_RESULTS = None
_CACHED_NC = None


def _u_perm():
    p = []
    for h in range(HEADS):
        p += list(range(h * HD, (h + 1) * HD))
        p += list(range(DIM + h * HD, DIM + (h + 1) * HD))
    return np.array(p)


def _bf16(a):
    return np.ascontiguousarray(a.astype(ml_dtypes.bfloat16))


def build_nc():
    nc = bacc.Bacc("TRN2", target_bir_lowering=False, debug=False,
                   enable_asserts=False, num_devices=N_CORES)
    xb = nc.dram_tensor("xb", [B, DIM, NF], BF16, kind="ExternalInput").ap()
    yb = nc.dram_tensor("yb", [B, DIM, NF], BF16, kind="ExternalInput").ap()
    wqk = nc.dram_tensor("wqk", [DIM, 2 * DIM], BF16, kind="ExternalInput").ap()
    aqk = nc.dram_tensor("aqk", [3, 128, 9], F32, kind="ExternalInput").ap()
    wv = nc.dram_tensor("wv", [DIM, DIM], BF16, kind="ExternalInput").ap()
    wvdw = nc.dram_tensor("wvdw", [9, DIM, DIM], BF16, kind="ExternalInput").ap()
    projth = nc.dram_tensor("projth", [HEADS, HD, DIM], BF16, kind="ExternalInput").ap()
    tempt = nc.dram_tensor("tempt", [HD, HEADS], F32, kind="ExternalInput").ap()
    eye = nc.dram_tensor("eye", [UD, UD], F32, kind="ExternalInput").ap()
    out = nc.dram_tensor("out", [B, DIM, NI], F32, kind="ExternalOutput").ap()

    with tile.TileContext(nc) as tc:
        with tc.tile_pool(name="wpool", bufs=1) as wp, \
             tc.tile_pool(name="xy", bufs=2) as xyp, \
             tc.tile_pool(name="mid", bufs=2) as midp, \
             tc.tile_pool(name="qk", bufs=1) as qkp, \
             tc.tile_pool(name="ut", bufs=2) as utp, \
             tc.tile_pool(name="vv", bufs=1) as vvp, \
             tc.tile_pool(name="small", bufs=1) as smp, \
             tc.tile_pool(name="ost", bufs=2) as ostp, \
             tc.tile_pool(name="psA", bufs=3, space="PSUM") as psA, \
             tc.tile_pool(name="psB", bufs=2, space="PSUM") as psB, \
             tc.tile_pool(name="psG", bufs=2, space="PSUM") as psG, \
             tc.tile_pool(name="psS", bufs=1, space="PSUM") as psS, \
             tc.tile_pool(name="dram", bufs=2, space="DRAM") as drp:

            # ---- persistent weights ----
            wqk_a = wp.tile([128, 2 * DIM], BF16, tag="wqk_a")
            wqk_b = wp.tile([64, 2 * DIM], BF16, tag="wqk_b")
            nc.sync.dma_start(wqk_a[:], wqk[0:128, :])
            nc.sync.dma_start(wqk_b[:], wqk[128:192, :])
            wv_a = wp.tile([128, DIM], BF16, tag="wv_a")
            wv_b = wp.tile([64, DIM], BF16, tag="wv_b")
            nc.sync.dma_start(wv_a[:], wv[0:128, :])
            nc.sync.dma_start(wv_b[:], wv[128:192, :])
            wvdw_a = wp.tile([128, 9, DIM], BF16, tag="wvdw_a")
            wvdw_b = wp.tile([64, 9, DIM], BF16, tag="wvdw_b")
            nc.sync.dma_start(wvdw_a[:], wvdw[:, 0:128, :].rearrange("t k m -> k t m"))
            nc.sync.dma_start(wvdw_b[:], wvdw[:, 128:192, :].rearrange("t k m -> k t m"))
            aqk_sb = wp.tile([128, 3, 9], F32, tag="aqk")
            nc.sync.dma_start(aqk_sb[:], aqk.rearrange("c k t -> k c t"))
            projth_sb = wp.tile([HD, HEADS, DIM], BF16, tag="projth")
            nc.sync.dma_start(projth_sb[:], projth.rearrange("h d f -> d h f"))
            tempt_sb = wp.tile([HD, HEADS], F32, tag="tempt")
            nc.sync.dma_start(tempt_sb[:], tempt[:])
            eye_sb = wp.tile([UD, UD], F32, tag="eye")
            nc.sync.dma_start(eye_sb[:], eye[:])

            # input column-chunk groups for streaming band loads
            GRP = 1024
            groups = [(g, min(g + GRP, NF)) for g in range(0, NF, GRP)]

            def qk_phase(b):
                """1x1 conv into guarded mid tiles + depthwise -> qkt."""
                qkt = []
                for ct in range(3):
                    mid = midp.tile([128, RIN, WG], BF16, tag="mid")
                    mid2 = midp.tile([128, RIN, W], BF16, tag="mid2")
                    nc.vector.memset(mid[:, :, 0:1], 0.0)
                    nc.vector.memset(mid[:, :, WG - 1:WG], 0.0)
                    mcol = slice(ct * 128, (ct + 1) * 128)
                    for (g0, g1) in groups:
                        x_a = xyp.tile([128, GRP], BF16, tag="band_a")
                        x_b = xyp.tile([64, GRP], BF16, tag="band_b")
                        nc.sync.dma_start(x_a[:, 0:g1 - g0], xb[b, 0:128, g0:g1])
                        nc.sync.dma_start(x_b[:, 0:g1 - g0], xb[b, 128:192, g0:g1])
                        for nb in range(g0 // NB, g1 // NB):
                            ns = slice(nb * NB - g0, (nb + 1) * NB - g0)
                            ps = psA.tile([128, NB], F32, tag="psA")
                            nc.tensor.matmul(ps[:], wqk_a[:, mcol], x_a[:, ns],
                                             start=True, stop=False)
                            nc.tensor.matmul(ps[:], wqk_b[:, mcol], x_b[:, ns],
                                             start=False, stop=True)
                            r = nb * 2
                            psv = ps[:].rearrange("p (r w) -> p r w", r=2)
                            nc.scalar.copy(mid[:, r:r + 2, 1:W + 1], psv)
                            nc.vector.tensor_copy(mid2[:, r:r + 2, :], psv)
                    # depthwise 3x3 -> q,k interior. tensor_scalar runs at 4x
                    # and tensor_tensor at 2x on DVE; the fused
                    # scalar_tensor_tensor is stuck at 1x.
                    qt = qkp.tile([128, ROWS, W], BF16, tag=f"qk{ct}")
                    dwt = qkp.tile([128, ROWS, W], BF16, tag="dwtmp")
                    nc.vector.tensor_scalar_mul(
                        qt[:], mid2[:, 1:1 + ROWS, :], aqk_sb[:, ct, 4:5])
                    for t, (dy, dx) in enumerate(TAPS):
                        if (dy, dx) == (0, 0):
                            continue
                        rs = slice(1 + dy, 1 + ROWS + dy)
                        if dx == 0:
                            src = mid2[:, rs, :]
                        else:
                            src = mid[:, rs, 1 + dx:1 + W + dx]
                        nc.vector.tensor_scalar_mul(
                            dwt[:], src, aqk_sb[:, ct, t:t + 1])
                        nc.vector.tensor_tensor(qt[:], dwt[:], qt[:], op=ADD)
                    qkt.append(qt)
                return qkt

            def v1_phase(b):
                v1a = midp.tile([128, RIN, WG], BF16, tag="mid")
                v1b = midp.tile([64, RIN, WG], BF16, tag="mid2")
                for t_ in (v1a, v1b):
                    nc.vector.memset(t_[:, :, 0:1], 0.0)
                    nc.vector.memset(t_[:, :, WG - 1:WG], 0.0)
                for (g0, g1) in groups:
                    y_a = xyp.tile([128, GRP], BF16, tag="band_a")
                    y_b = xyp.tile([64, GRP], BF16, tag="band_b")
                    nc.sync.dma_start(y_a[:, 0:g1 - g0], yb[b, 0:128, g0:g1])
                    nc.sync.dma_start(y_b[:, 0:g1 - g0], yb[b, 128:192, g0:g1])
                    for nb in range(g0 // NB, g1 // NB):
                        ns = slice(nb * NB - g0, (nb + 1) * NB - g0)
                        ps = psA.tile([128, NB], F32, tag="psA")
                        ps2 = psB.tile([64, NB], F32, tag="psB")
                        nc.tensor.matmul(ps[:], wv_a[:, 0:128], y_a[:, ns],
                                         start=True, stop=False)
                        nc.tensor.matmul(ps[:], wv_b[:, 0:128], y_b[:, ns],
                                         start=False, stop=True)
                        nc.tensor.matmul(ps2[:], wv_a[:, 128:192], y_a[:, ns],
                                         start=True, stop=False)
                        nc.tensor.matmul(ps2[:], wv_b[:, 128:192], y_b[:, ns],
                                         start=False, stop=True)
                        r = nb * 2
                        nc.scalar.copy(v1a[:, r:r + 2, 1:W + 1],
                                       ps[:].rearrange("p (r w) -> p r w", r=2))
                        nc.scalar.copy(v1b[:, r:r + 2, 1:W + 1],
                                       ps2[:].rearrange("p (r w) -> p r w", r=2))
                return v1a, v1b

            def vdw_phase(b, v1a, v1b):
                v0 = vvp.tile([128, NI], BF16, tag="v0")
                vB = vvp.tile([64, NI], BF16, tag="vB")
                for nb in range(NI // NB):
                    ns = slice(nb * NB, (nb + 1) * NB)
                    r = nb * 2
                    psv = psA.tile([128, NB], F32, tag="psA")
                    psv2 = psB.tile([64, NB], F32, tag="psB")
                    for t, (dy, dx) in enumerate(TAPS):
                        rs = slice(1 + r + dy, 3 + r + dy)
                        cs = slice(1 + dx, 1 + W + dx)
                        rhsA = v1a[:, rs, cs]
                        rhsB = v1b[:, rs, cs]
                        nc.tensor.matmul(psv[:], wvdw_a[:, t, 0:128], rhsA,
                                         start=(t == 0), stop=False)
                        nc.tensor.matmul(psv[:], wvdw_b[:, t, 0:128], rhsB,
                                         start=False, stop=(t == 8))
                        nc.tensor.matmul(psv2[:], wvdw_a[:, t, 128:192], rhsA,
                                         start=(t == 0), stop=False)
                        nc.tensor.matmul(psv2[:], wvdw_b[:, t, 128:192], rhsB,
                                         start=False, stop=(t == 8))
                    nc.scalar.copy(v0[:, ns], psv[:])
                    nc.scalar.copy(vB[:, ns], psv2[:])
                return v0, vB

            def gram_ar_phase(b, qkt):
                gu_all = smp.tile([UD, HEADS, UD], F32, tag="gu_all")
                n_bands = NI // 2048
                for h in range(HEADS):
                    gps = psG.tile([UD, UD], F32, tag="psG")
                    for band in range(n_bands):
                        rsl = slice(band * 8, (band + 1) * 8)
                        ut = utp.tile([128, 16, UD], BF16, tag="ut")
                        off = 0
                        for (ct, lo, hi) in HEAD_SEGS[h]:
                            nc.sync.dma_start_transpose(
                                ut[:, :, off:off + hi - lo],
                                qkt[ct][lo:hi, rsl, :])
                            off += hi - lo
                        for c in range(16):
                            nc.tensor.matmul(
                                gps[:], ut[:, c, :], ut[:, c, :],
                                start=(band == 0 and c == 0),
                                stop=(band == n_bands - 1 and c == 15))
                    nc.vector.tensor_copy(gu_all[:, h, :], gps[:])
                gu_in = drp.tile([UD, HEADS * UD], F32, tag="gu_in")
                gu_out = drp.tile([UD, HEADS * UD], F32, tag="gu_out")
                nc.sync.dma_start(gu_in[:], gu_all[:].rearrange("p h d -> p (h d)"))
                nc.gpsimd.collective_compute(
                    "AllReduce", ADD,
                    replica_groups=[list(range(N_CORES))],
                    ins=[gu_in.opt()], outs=[gu_out.opt()])
                gur = smp.tile([UD, HEADS, UD], F32, tag="gur")
                nc.sync.dma_start(gur[:].rearrange("p h d -> p (h d)"), gu_out[:])
                return gur

            def post_phase(b, gur):
                tmp = smp.tile([UD, UD], F32, tag="tmp")
                nsq = smp.tile([UD, HEADS], F32, tag="nsq")
                ksqr = smp.tile([1, HEADS, UD], F32, tag="ksqr")
                for h in range(HEADS):
                    # masked diag of the whole [96,96] head block (partition
                    # bases must be 32-aligned, so work on the full block)
                    nc.vector.tensor_tensor(
                        tmp[:], gur[:, h, :], eye_sb[:], op=MULT)
                    nc.vector.tensor_reduce(
                        nsq[:, h:h + 1], tmp[:],
                        axis=mybir.AxisListType.X, op=ADD)
                    nc.gpsimd.tensor_reduce(
                        ksqr[:, h, :], tmp[:],
                        axis=mybir.AxisListType.C, op=ADD)
                rn = smp.tile([HD, HEADS], F32, tag="rn")
                inv = smp.tile([HD, HEADS], F32, tag="inv")
                nc.scalar.sqrt(rn[:], nsq[0:HD, :])
                nc.vector.reciprocal(inv[:], rn[:])
                invq = smp.tile([HD, HEADS], F32, tag="invq")
                nc.vector.tensor_tensor(invq[:], inv[:], tempt_sb[:], op=MULT)
                krn = smp.tile([1, HEADS, UD], F32, tag="krn")
                invkr = smp.tile([1, HEADS, UD], F32, tag="invkr")
                nc.scalar.sqrt(krn[:], ksqr[:])
                nc.vector.reciprocal(invkr[:], krn[:])
                mst = smp.tile([HD, HEADS, DIM], BF16, tag="mst")
                for h in range(HEADS):
                    bc = smp.tile([HD, HD], F32, tag="bc")
                    nc.gpsimd.partition_broadcast(bc[:], invkr[:, h, HD:UD])
                    lg = smp.tile([HD, HD], F32, tag="lg")
                    nc.vector.tensor_scalar_mul(
                        lg[:], gur[0:HD, h, HD:UD], invq[:, h:h + 1])
                    nc.vector.tensor_tensor(lg[:], lg[:], bc[:], op=MULT)
                    nm = smp.tile([HD, 1], F32, tag="nm")
                    nc.vector.tensor_reduce(
                        nm[:], lg[:], axis=mybir.AxisListType.X,
                        op=mybir.AluOpType.max, negate=True)
                    ex = smp.tile([HD, HD], F32, tag="ex")
                    nc.scalar.activation(ex[:], lg[:],
                                         mybir.ActivationFunctionType.Exp,
                                         bias=nm[:], scale=1.0)
                    sm = smp.tile([HD, 1], F32, tag="sm")
                    nc.vector.tensor_reduce(sm[:], ex[:],
                                            axis=mybir.AxisListType.X, op=ADD)
                    rs_ = smp.tile([HD, 1], F32, tag="rs_")
                    nc.vector.reciprocal(rs_[:], sm[:])
                    at = smp.tile([HD, HD], BF16, tag="at")
                    nc.vector.tensor_scalar_mul(at[:], ex[:], rs_[:])
                    mps = psS.tile([HD, DIM], F32, tag="psS")
                    nc.tensor.matmul(mps[:], at[:], projth_sb[:, h, :],
                                     start=True, stop=True)
                    nc.vector.tensor_copy(mst[:, h, :], mps[:])
                peff_a = smp.tile([128, DIM], BF16, tag="peff_a")
                peff_b = smp.tile([64, DIM], BF16, tag="peff_b")
                nc.sync.dma_start(peff_a[0:48, :], mst[:, 0, :])
                nc.sync.dma_start(peff_a[48:96, :], mst[:, 1, :])
                nc.sync.dma_start(peff_a[96:128, :], mst[0:32, 2, :])
                nc.sync.dma_start(peff_b[0:16, :], mst[32:48, 2, :])
                nc.sync.dma_start(peff_b[16:64, :], mst[:, 3, :])
                return peff_a, peff_b

            def final_phase(b, peff_a, peff_b, v0, vB):
                for (f0, f1) in ((0, 128), (128, 192)):
                    for nb in range(NI // NB):
                        ns = slice(nb * NB, (nb + 1) * NB)
                        if f1 - f0 == 128:
                            psf = psA.tile([128, NB], F32, tag="psA")
                        else:
                            psf = psB.tile([64, NB], F32, tag="psB")
                        nc.tensor.matmul(psf[:], peff_a[:, f0:f1], v0[:, ns],
                                         start=True, stop=False)
                        nc.tensor.matmul(psf[:], peff_b[:, f0:f1], vB[:, ns],
                                         start=False, stop=True)
                        ost = ostp.tile([f1 - f0, NB], F32, tag="ost")
                        nc.scalar.copy(ost[:], psf[:])
                        nc.sync.dma_start(out[b, f0:f1, ns], ost[:])

            # ---- global schedule ----
            # engine streams are in-order, so emission order sets per-engine
            # instruction order. batch b+1's conv work is emitted before
            # batch b's final so PE covers the AllReduce + softmax latency.
            qkt0 = qk_phase(0)
            v1a0, v1b0 = v1_phase(0)
            v00, vB0 = vdw_phase(0, v1a0, v1b0)
            gur0 = gram_ar_phase(0, qkt0)
            peff_a0, peff_b0 = post_phase(0, gur0)
            qkt1 = qk_phase(1)
            v1a1, v1b1 = v1_phase(1)
            final_phase(0, peff_a0, peff_b0, v00, vB0)
            v01, vB1 = vdw_phase(1, v1a1, v1b1)
            gur1 = gram_ar_phase(1, qkt1)
            peff_a1, peff_b1 = post_phase(1, gur1)
            final_phase(1, peff_a1, peff_b1, v01, vB1)

    nc.finalize()
    return nc


def _host_prep(inputs):
    x = np.asarray(inputs["x"], dtype=np.float32)
    y = np.asarray(inputs["y"], dtype=np.float32)
    qk_w = np.asarray(inputs["qk_w"], dtype=np.float32)[:, :, 0, 0]
    qk_dw = np.asarray(inputs["qk_dw_w"], dtype=np.float32)[:, 0]
    v_w = np.asarray(inputs["v_w"], dtype=np.float32)[:, :, 0, 0]
    v_dw = np.asarray(inputs["v_dw_w"], dtype=np.float32)
    proj = np.asarray(inputs["proj_w"], dtype=np.float32)[:, :, 0, 0]
    temp = np.asarray(inputs["temperature"], dtype=np.float32).reshape(HEADS)

    perm = _u_perm()
    wqk_l = _bf16(qk_w[perm].T)                              # [192, 384]
    aqk_t = np.ascontiguousarray(qk_dw[perm].reshape(3, 128, 9).astype(np.float32))
    wv_l = _bf16(v_w.T)                                      # [192, 192]
    wvdw_l = _bf16(np.stack(
        [v_dw[:, :, dy + 1, dx + 1].T for dy, dx in TAPS]))  # [9,192,192]
    projth = _bf16(np.stack(
        [proj[:, h * HD:(h + 1) * HD].T for h in range(HEADS)]))  # [4,48,192]
    tempt = np.ascontiguousarray(
        np.broadcast_to(temp[None, :], (HD, HEADS)).astype(np.float32))
    eye = np.eye(UD, dtype=np.float32)

    # halo-padded row bands per core, bf16
    xp = np.pad(x, ((0, 0), (0, 0), (1, 1), (0, 0)))
    yp = np.pad(y, ((0, 0), (0, 0), (1, 1), (0, 0)))
    shared = dict(wqk=wqk_l, aqk=aqk_t, wv=wv_l, wvdw=wvdw_l,
                  projth=projth, tempt=tempt, eye=eye)
    in_maps = []
    for c in range(N_CORES):
        rs = slice(c * ROWS, c * ROWS + RIN)
        in_maps.append(dict(
            xb=_bf16(xp[:, :, rs]).reshape(B, DIM, NF),
            yb=_bf16(yp[:, :, rs]).reshape(B, DIM, NF),
            **shared))
    return in_maps


def kernel(**inputs):
    global LAST_RESULTS, _CACHED_NC
    in_maps = _host_prep(inputs)
    if _CACHED_NC is None:
        _CACHED_NC = build_nc()
    res = run_bass_kernel_spmd(
        _CACHED_NC, in_maps, core_ids=list(range(N_CORES)))
    LAST_RESULTS = res
    out = np.empty((B, DIM, H, W), np.float32)
    for c in range(N_CORES):
        band = res.results[c]["out"].reshape(B, DIM, ROWS, W)
        out[:, :, c * ROWS:(c + 1) * ROWS] = band
    return out


# revision 15
# speedup vs baseline: 1.1423x; 1.0553x over previous
"""Channel-attention (XCA-style) Trainium2 kernel, 8-core SPMD.

Sharding: spatial row-bands (32 rows/core + 1-row halo), both batches on
every core. Cross-core coupling is only the per-(batch,head) Gram matrix
[q;k][q;k]^T and the q/k squared norms, all-reduced as one tiny [96,384]
fp32 buffer per batch. Softmax + proj-folding is recomputed redundantly
per core; the attention@v + proj matmul is fused into a single
PeffT.T @ v matmul per batch.

Pipeline per core, per batch:
  x band --1x1(PE)--> mid(+guard cols) --depthwise 3x3(DVE FMA)--> q,k
  q,k --DMA-xbar-transpose--> uT chunks --PE Gram--> Gu psum --AllReduce
  y band --1x1(PE)--> v1(+guards) --9-tap full 3x3 (PE)--> v
  post-AR: norms/softmax/Peff (tiny, redundant) -> PeffT
  final: PeffT.T @ v -> out band (fp32)
"""
import os
import sys

sys.path.insert(0, '/opt/trn_rl_repo')

import numpy as np
import ml_dtypes

import concourse.bass as bass
import concourse.bacc as bacc
import concourse.tile as tile
import concourse.mybir as mybir
from concourse.bass_utils import run_bass_kernel_spmd

BF16 = mybir.dt.bfloat16
F32 = mybir.dt.float32
ADD = mybir.AluOpType.add
MULT = mybir.AluOpType.mult

N_CORES = 8
DIM = 192
HEADS = 4
HD = DIM // HEADS            # 48
UD = 2 * HD                  # 96 channels per head in u=[q_h;k_h] order
B = 2
H = 256
W = 256
ROWS = H // N_CORES          # 32 output rows per core
RIN = ROWS + 2               # input band rows (halo)
WG = W + 2                   # guarded width
NI = ROWS * W                # 8192 interior positions per batch
NF = RIN * W                 # 8704 band positions per batch
NB = 512                     # matmul N-chunk
TAPS = [(dy, dx) for dy in (-1, 0, 1) for dx in (-1, 0, 1)]

# segments of qk tiles covering u-channels [96h, 96h+96) : (tile, lo, hi)
# XBAR transpose DMA sources starting at partition > 0 are limited to 32
# partitions, so non-zero-base segments are split into 32-row chunks.
def _segs(raw):
    out = []
    for (ct, lo, hi) in raw:
        if lo == 0:
            out.append((ct, lo, hi))
        else:
            out += [(ct, s, min(s + 32, hi)) for s in range(lo, hi, 32)]
    return out


HEAD_SEGS = {
    0: _segs([(0, 0, 96)]),
    1: _segs([(0, 96, 128), (1, 0, 64)]),
    2: _segs([(1, 64, 128), (2, 0, 32)]),
    3: _segs([(2, 32, 128)]),
}

LAST_RESULTS = None
_CACHED_NC = None


def _u_perm():
    p = []
    for h in range(HEADS):
        p += list(range(h * HD, (h + 1) * HD))
        p += list(range(DIM + h * HD, DIM + (h + 1) * HD))
    return np.array(p)


def _bf16(a):
    return np.ascontiguousarray(a.astype(ml_dtypes.bfloat16))


def build_nc():
    nc = bacc.Bacc("TRN2", target_bir_lowering=False, debug=False,
                   enable_asserts=False, num_devices=N_CORES)
    xb = nc.dram_tensor("xb", [B, DIM, NF], BF16, kind="ExternalInput").ap()
    yb = nc.dram_tensor("yb", [B, DIM, NF], BF16, kind="ExternalInput").ap()
    wqk = nc.dram_tensor("wqk", [DIM, 2 * DIM], BF16, kind="ExternalInput").ap()
    aqk = nc.dram_tensor("aqk", [3, 128, 9], F32, kind="ExternalInput").ap()
    wv = nc.dram_tensor("wv", [DIM, DIM], BF16, kind="ExternalInput").ap()
    wvdw = nc.dram_tensor("wvdw", [9, DIM, DIM], BF16, kind="ExternalInput").ap()
    projth = nc.dram_tensor("projth", [HEADS, HD, DIM], BF16, kind="ExternalInput").ap()
    tempt = nc.dram_tensor("tempt", [HD, HEADS], F32, kind="ExternalInput").ap()
    eye = nc.dram_tensor("eye", [UD, UD], F32, kind="ExternalInput").ap()
    out = nc.dram_tensor("out", [B, DIM, NI], F32, kind="ExternalOutput").ap()

    with tile.TileContext(nc) as tc:
        with tc.tile_pool(name="wpool", bufs=1) as wp, \
             tc.tile_pool(name="xy", bufs=2) as xyp, \
             tc.tile_pool(name="mid", bufs=2) as midp, \
             tc.tile_pool(name="qk", bufs=1) as qkp, \
             tc.tile_pool(name="ut", bufs=2) as utp, \
             tc.tile_pool(name="vv", bufs=1) as vvp, \
             tc.tile_pool(name="small", bufs=1) as smp, \
             tc.tile_pool(name="ost", bufs=2) as ostp, \
             tc.tile_pool(name="psA", bufs=3, space="PSUM") as psA, \
             tc.tile_pool(name="psB", bufs=2, space="PSUM") as psB, \
             tc.tile_pool(name="psG", bufs=2, space="PSUM") as psG, \
             tc.tile_pool(name="psS", bufs=1, space="PSUM") as psS, \
             tc.tile_pool(name="dram", bufs=2, space="DRAM") as drp:

            # ---- persistent weights ----
            wqk_a = wp.tile([128, 2 * DIM], BF16, tag="wqk_a")
            wqk_b = wp.tile([64, 2 * DIM], BF16, tag="wqk_b")
            nc.sync.dma_start(wqk_a[:], wqk[0:128, :])
            nc.sync.dma_start(wqk_b[:], wqk[128:192, :])
            wv_a = wp.tile([128, DIM], BF16, tag="wv_a")
            wv_b = wp.tile([64, DIM], BF16, tag="wv_b")
            nc.sync.dma_start(wv_a[:], wv[0:128, :])
            nc.sync.dma_start(wv_b[:], wv[128:192, :])
            wvdw_a = wp.tile([128, 9, DIM], BF16, tag="wvdw_a")
            wvdw_b = wp.tile([64, 9, DIM], BF16, tag="wvdw_b")
            nc.sync.dma_start(wvdw_a[:], wvdw[:, 0:128, :].rearrange("t k m -> k t m"))
            nc.sync.dma_start(wvdw_b[:], wvdw[:, 128:192, :].rearrange("t k m -> k t m"))
            aqk_sb = wp.tile([128, 3, 9], F32, tag="aqk")
            nc.sync.dma_start(aqk_sb[:], aqk.rearrange("c k t -> k c t"))
            projth_sb = wp.tile([HD, HEADS, DIM], BF16, tag="projth")
            nc.sync.dma_start(projth_sb[:], projth.rearrange("h d f -> d h f"))
            tempt_sb = wp.tile([HD, HEADS], F32, tag="tempt")
            nc.sync.dma_start(tempt_sb[:], tempt[:])
            eye_sb = wp.tile([UD, UD], F32, tag="eye")
            nc.sync.dma_start(eye_sb[:], eye[:])
            ones_sb = wp.tile([UD, UD], F32, tag="ones")
            nc.vector.memset(ones_sb[:], 1.0)

            # input column-chunk groups for streaming band loads
            GRP = 1024
            groups = [(g, min(g + GRP, NF)) for g in range(0, NF, GRP)]

            def qk_phase(b, post_cb=None):
                """1x1 conv into guarded mid tiles + depthwise -> qkt."""
                qkt = []
                for ct in range(3):
                    mid = midp.tile([128, RIN, WG], BF16, tag="mid")
                    mid2 = midp.tile([128, RIN, W], BF16, tag="mid2")
                    nc.gpsimd.memset(mid[:, :, 0:1], 0.0)
                    nc.gpsimd.memset(mid[:, :, WG - 1:WG], 0.0)
                    mcol = slice(ct * 128, (ct + 1) * 128)
                    for (g0, g1) in groups:
                        x_a = xyp.tile([128, GRP], BF16, tag="band_a")
                        x_b = xyp.tile([64, GRP], BF16, tag="band_b")
                        nc.sync.dma_start(x_a[:, 0:g1 - g0], xb[b, 0:128, g0:g1])
                        nc.sync.dma_start(x_b[:, 0:g1 - g0], xb[b, 128:192, g0:g1])
                        for nb in range(g0 // NB, g1 // NB):
                            ns = slice(nb * NB - g0, (nb + 1) * NB - g0)
                            ps = psA.tile([128, NB], F32, tag="psA")
                            nc.tensor.matmul(ps[:], wqk_a[:, mcol], x_a[:, ns],
                                             start=True, stop=False)
                            nc.tensor.matmul(ps[:], wqk_b[:, mcol], x_b[:, ns],
                                             start=False, stop=True)
                            r = nb * 2
                            psv = ps[:].rearrange("p (r w) -> p r w", r=2)
                            nc.scalar.copy(mid[:, r:r + 2, 1:W + 1], psv)
                            nc.scalar.copy(mid2[:, r:r + 2, :], psv)
                    # depthwise 3x3 -> q,k interior. tensor_scalar runs at 4x
                    # and tensor_tensor at 2x on DVE; the fused
                    # scalar_tensor_tensor is stuck at 1x.
                    qt = qkp.tile([128, ROWS, W], BF16, tag=f"qk{ct}")
                    dwt = qkp.tile([128, ROWS, W], BF16, tag="dwtmp")
                    nc.vector.tensor_scalar_mul(
                        qt[:], mid2[:, 1:1 + ROWS, :], aqk_sb[:, ct, 4:5])
                    for t, (dy, dx) in enumerate(TAPS):
                        if (dy, dx) == (0, 0):
                            continue
                        rs = slice(1 + dy, 1 + ROWS + dy)
                        if dx == 0:
                            src = mid2[:, rs, :]
                        else:
                            src = mid[:, rs, 1 + dx:1 + W + dx]
                        nc.vector.tensor_scalar_mul(
                            dwt[:], src, aqk_sb[:, ct, t:t + 1])
                        nc.vector.tensor_tensor(qt[:], dwt[:], qt[:], op=ADD)
                    qkt.append(qt)
                    if ct == 0 and post_cb is not None:
                        post_cb()
                return qkt

            def v1_phase(b):
                v1a = midp.tile([128, RIN, WG], BF16, tag="mid")
                v1b = midp.tile([64, RIN, WG], BF16, tag="mid2")
                for t_ in (v1a, v1b):
                    nc.gpsimd.memset(t_[:, :, 0:1], 0.0)
                    nc.gpsimd.memset(t_[:, :, WG - 1:WG], 0.0)
                for (g0, g1) in groups:
                    y_a = xyp.tile([128, GRP], BF16, tag="band_a")
                    y_b = xyp.tile([64, GRP], BF16, tag="band_b")
                    nc.sync.dma_start(y_a[:, 0:g1 - g0], yb[b, 0:128, g0:g1])
                    nc.sync.dma_start(y_b[:, 0:g1 - g0], yb[b, 128:192, g0:g1])
                    for nb in range(g0 // NB, g1 // NB):
                        ns = slice(nb * NB - g0, (nb + 1) * NB - g0)
                        ps = psA.tile([128, NB], F32, tag="psA")
                        ps2 = psB.tile([64, NB], F32, tag="psB")
                        nc.tensor.matmul(ps[:], wv_a[:, 0:128], y_a[:, ns],
                                         start=True, stop=False)
                        nc.tensor.matmul(ps[:], wv_b[:, 0:128], y_b[:, ns],
                                         start=False, stop=True)
                        nc.tensor.matmul(ps2[:], wv_a[:, 128:192], y_a[:, ns],
                                         start=True, stop=False)
                        nc.tensor.matmul(ps2[:], wv_b[:, 128:192], y_b[:, ns],
                                         start=False, stop=True)
                        r = nb * 2
                        nc.scalar.copy(v1a[:, r:r + 2, 1:W + 1],
                                       ps[:].rearrange("p (r w) -> p r w", r=2))
                        nc.scalar.copy(v1b[:, r:r + 2, 1:W + 1],
                                       ps2[:].rearrange("p (r w) -> p r w", r=2))
                return v1a, v1b

            def vdw_phase(b, v1a, v1b):
                v0 = vvp.tile([128, NI], BF16, tag="v0")
                vB = vvp.tile([64, NI], BF16, tag="vB")
                for g in range(0, NI // NB, 2):
                    nbs = [g, g + 1]
                    psvs = [psA.tile([128, NB], F32, tag="psA", name="psv") for _ in nbs]
                    psv2s = [psB.tile([64, NB], F32, tag="psB", name="psv2") for _ in nbs]
                    for t, (dy, dx) in enumerate(TAPS):
                        cs = slice(1 + dx, 1 + W + dx)
                        rows = [slice(1 + nb * 2 + dy, 3 + nb * 2 + dy)
                                for nb in nbs]
                        for j, nb in enumerate(nbs):
                            nc.tensor.matmul(
                                psvs[j][:], wvdw_a[:, t, 0:128],
                                v1a[:, rows[j], cs],
                                start=(t == 0), stop=False)
                        for j, nb in enumerate(nbs):
                            nc.tensor.matmul(
                                psvs[j][:], wvdw_b[:, t, 0:128],
                                v1b[:, rows[j], cs],
                                start=False, stop=(t == 8))
                        for j, nb in enumerate(nbs):
                            nc.tensor.matmul(
                                psv2s[j][:], wvdw_a[:, t, 128:192],
                                v1a[:, rows[j], cs],
                                start=(t == 0), stop=False)
                        for j, nb in enumerate(nbs):
                            nc.tensor.matmul(
                                psv2s[j][:], wvdw_b[:, t, 128:192],
                                v1b[:, rows[j], cs],
                                start=False, stop=(t == 8))
                    for j, nb in enumerate(nbs):
                        ns = slice(nb * NB, (nb + 1) * NB)
                        nc.scalar.copy(v0[:, ns], psvs[j][:])
                        nc.scalar.copy(vB[:, ns], psv2s[j][:])
                return v0, vB

            def gram_ar_phase(b, qkt):
                gu_all = smp.tile([UD, HEADS, UD], F32, tag="gu_all")
                n_bands = NI // 2048
                for h in range(HEADS):
                    gps = psG.tile([UD, UD], F32, tag="psG")
                    for band in range(n_bands):
                        rsl = slice(band * 8, (band + 1) * 8)
                        ut = utp.tile([128, 16, UD], BF16, tag="ut")
                        off = 0
                        for (ct, lo, hi) in HEAD_SEGS[h]:
                            nc.sync.dma_start_transpose(
                                ut[:, :, off:off + hi - lo],
                                qkt[ct][lo:hi, rsl, :])
                            off += hi - lo
                        for c in range(16):
                            nc.tensor.matmul(
                                gps[:], ut[:, c, :], ut[:, c, :],
                                start=(band == 0 and c == 0),
                                stop=(band == n_bands - 1 and c == 15))
                    nc.vector.tensor_copy(gu_all[:, h, :], gps[:])
                gu_in = drp.tile([UD, HEADS * UD], F32, tag="gu_in")
                gu_out = drp.tile([UD, HEADS * UD], F32, tag="gu_out")
                nc.sync.dma_start(gu_in[:], gu_all[:].rearrange("p h d -> p (h d)"))
                nc.gpsimd.collective_compute(
                    "AllReduce", ADD,
                    replica_groups=[list(range(N_CORES))],
                    ins=[gu_in.opt()], outs=[gu_out.opt()])
                gur = smp.tile([UD, HEADS, UD], F32, tag="gur")
                nc.sync.dma_start(gur[:].rearrange("p h d -> p (h d)"), gu_out[:])
                return gur

            def post_phase(b, gur):
                tmp = smp.tile([UD, UD], F32, tag="tmp")
                nsq = smp.tile([UD, HEADS], F32, tag="nsq")
                ksqr = smp.tile([1, HEADS, UD], F32, tag="ksqr")
                for h in range(HEADS):
                    # masked diag of the whole [96,96] head block (partition
                    # bases must be 32-aligned, so work on the full block)
                    nc.vector.tensor_tensor(
                        tmp[:], gur[:, h, :], eye_sb[:], op=MULT)
                    nc.vector.tensor_reduce(
                        nsq[:, h:h + 1], tmp[:],
                        axis=mybir.AxisListType.X, op=ADD)
                    kps = psS.tile([1, UD], F32, tag="psS")
                    nc.tensor.matmul(kps[:], ones_sb[:, 0:1], tmp[:],
                                     start=True, stop=True)
                    nc.vector.tensor_copy(ksqr[:, h, :], kps[:])
                rn = smp.tile([HD, HEADS], F32, tag="rn")
                inv = smp.tile([HD, HEADS], F32, tag="inv")
                nc.scalar.sqrt(rn[:], nsq[0:HD, :])
                nc.vector.reciprocal(inv[:], rn[:])
                invq = smp.tile([HD, HEADS], F32, tag="invq")
                nc.vector.tensor_tensor(invq[:], inv[:], tempt_sb[:], op=MULT)
                krn = smp.tile([1, HEADS, UD], F32, tag="krn")
                invkr = smp.tile([1, HEADS, UD], F32, tag="invkr")
                nc.scalar.sqrt(krn[:], ksqr[:])
                nc.vector.reciprocal(invkr[:], krn[:])
                mst = smp.tile([HD, HEADS, DIM], BF16, tag="mst")
                for h in range(HEADS):
                    bc = smp.tile([HD, HD], F32, tag="bc")
                    bps = psS.tile([HD, HD], F32, tag="psS")
                    nc.tensor.matmul(bps[:], ones_sb[0:1, 0:HD],
                                     invkr[:, h, HD:UD], start=True, stop=True)
                    nc.vector.tensor_copy(bc[:], bps[:])
                    lg = smp.tile([HD, HD], F32, tag="lg")
                    nc.vector.tensor_scalar_mul(
                        lg[:], gur[0:HD, h, HD:UD], invq[:, h:h + 1])
                    nc.vector.tensor_tensor(lg[:], lg[:], bc[:], op=MULT)
                    nm = smp.tile([HD, 1], F32, tag="nm")
                    nc.vector.tensor_reduce(
                        nm[:], lg[:], axis=mybir.AxisListType.X,
                        op=mybir.AluOpType.max, negate=True)
                    ex = smp.tile([HD, HD], F32, tag="ex")
                    nc.scalar.activation(ex[:], lg[:],
                                         mybir.ActivationFunctionType.Exp,
                                         bias=nm[:], scale=1.0)
                    sm = smp.tile([HD, 1], F32, tag="sm")
                    nc.vector.tensor_reduce(sm[:], ex[:],
                                            axis=mybir.AxisListType.X, op=ADD)
                    rs_ = smp.tile([HD, 1], F32, tag="rs_")
                    nc.vector.reciprocal(rs_[:], sm[:])
                    at = smp.tile([HD, HD], BF16, tag="at")
                    nc.vector.tensor_scalar_mul(at[:], ex[:], rs_[:])
                    mps = psS.tile([HD, DIM], F32, tag="psS")
                    nc.tensor.matmul(mps[:], at[:], projth_sb[:, h, :],
                                     start=True, stop=True)
                    nc.vector.tensor_copy(mst[:, h, :], mps[:])
                peff_a = smp.tile([128, DIM], BF16, tag="peff_a")
                peff_b = smp.tile([64, DIM], BF16, tag="peff_b")
                nc.sync.dma_start(peff_a[0:48, :], mst[:, 0, :])
                nc.sync.dma_start(peff_a[48:96, :], mst[:, 1, :])
                nc.sync.dma_start(peff_a[96:128, :], mst[0:32, 2, :])
                nc.sync.dma_start(peff_b[0:16, :], mst[32:48, 2, :])
                nc.sync.dma_start(peff_b[16:64, :], mst[:, 3, :])
                return peff_a, peff_b

            def final_phase(b, peff_a, peff_b, v0, vB):
                for (f0, f1) in ((0, 128), (128, 192)):
                    for nb in range(NI // NB):
                        ns = slice(nb * NB, (nb + 1) * NB)
                        if f1 - f0 == 128:
                            psf = psA.tile([128, NB], F32, tag="psA")
                        else:
                            psf = psB.tile([64, NB], F32, tag="psB")
                        nc.tensor.matmul(psf[:], peff_a[:, f0:f1], v0[:, ns],
                                         start=True, stop=False)
                        nc.tensor.matmul(psf[:], peff_b[:, f0:f1], vB[:, ns],
                                         start=False, stop=True)
                        ost = ostp.tile([f1 - f0, NB], F32, tag="ost")
                        nc.scalar.copy(ost[:], psf[:])
                        nc.sync.dma_start(out[b, f0:f1, ns], ost[:])

            # ---- global schedule ----
            # engine streams are in-order, so emission order sets per-engine
            # instruction order. batch b+1's conv work is emitted before
            # batch b's final so PE covers the AllReduce + softmax latency.
            qkt0 = qk_phase(0)
            v1a0, v1b0 = v1_phase(0)
            v00, vB0 = vdw_phase(0, v1a0, v1b0)
            gur0 = gram_ar_phase(0, qkt0)
            peff0 = []
            qkt1 = qk_phase(1, post_cb=lambda: peff0.extend(post_phase(0, gur0)))
            v1a1, v1b1 = v1_phase(1)
            final_phase(0, peff0[0], peff0[1], v00, vB0)
            v01, vB1 = vdw_phase(1, v1a1, v1b1)
            gur1 = gram_ar_phase(1, qkt1)
            peff_a1, peff_b1 = post_phase(1, gur1)
            final_phase(1, peff_a1, peff_b1, v01, vB1)

    nc.finalize()
    return nc


def _host_prep(inputs):
    x = np.asarray(inputs["x"], dtype=np.float32)
    y = np.asarray(inputs["y"], dtype=np.float32)
    qk_w = np.asarray(inputs["qk_w"], dtype=np.float32)[:, :, 0, 0]
    qk_dw = np.asarray(inputs["qk_dw_w"], dtype=np.float32)[:, 0]
    v_w = np.asarray(inputs["v_w"], dtype=np.float32)[:, :, 0, 0]
    v_dw = np.asarray(inputs["v_dw_w"], dtype=np.float32)
    proj = np.asarray(inputs["proj_w"], dtype=np.float32)[:, :, 0, 0]
    temp = np.asarray(inputs["temperature"], dtype=np.float32).reshape(HEADS)

    perm = _u_perm()
    wqk_l = _bf16(qk_w[perm].T)                              # [192, 384]
    aqk_t = np.ascontiguousarray(qk_dw[perm].reshape(3, 128, 9).astype(np.float32))
    wv_l = _bf16(v_w.T)                                      # [192, 192]
    wvdw_l = _bf16(np.stack(
        [v_dw[:, :, dy + 1, dx + 1].T for dy, dx in TAPS]))  # [9,192,192]
    projth = _bf16(np.stack(
        [proj[:, h * HD:(h + 1) * HD].T for h in range(HEADS)]))  # [4,48,192]
    tempt = np.ascontiguousarray(
        np.broadcast_to(temp[None, :], (HD, HEADS)).astype(np.float32))
    eye = np.eye(UD, dtype=np.float32)

    # halo-padded row bands per core, bf16
    xp = np.pad(x, ((0, 0), (0, 0), (1, 1), (0, 0)))
    yp = np.pad(y, ((0, 0), (0, 0), (1, 1), (0, 0)))
    shared = dict(wqk=wqk_l, aqk=aqk_t, wv=wv_l, wvdw=wvdw_l,
                  projth=projth, tempt=tempt, eye=eye)
    in_maps = []
    for c in range(N_CORES):
        rs = slice(c * ROWS, c * ROWS + RIN)
        in_maps.append(dict(
            xb=_bf16(xp[:, :, rs]).reshape(B, DIM, NF),
            yb=_bf16(yp[:, :, rs]).reshape(B, DIM, NF),
            **shared))
    return in_maps


def kernel(**inputs):
    global LAST_RESULTS, _CACHED_NC
    in_maps = _host_prep(inputs)
    if _CACHED_NC is None:
        _CACHED_NC = build_nc()
    res = run_bass_kernel_spmd(
        _CACHED_NC, in_maps, core_ids=list(range(N_CORES)))
    LAST_RESULTS = res
    out = np.empty((B, DIM, H, W), np.float32)
    for c in range(N_CORES):
        band = res.results[c]["out"].reshape(B, DIM, ROWS, W)
        out[:, :, c * ROWS:(c + 1) * ROWS] = band
    return out
